# revision 19
# baseline (speedup 1.0000x reference)
"""Fused multi-head attention (2 heads, RoPE-across-heads) on 8 Trainium2 NeuronCores.

Reference computation (per batch b of 4, seq 2048, dim 2048):
    qkv = x @ wqkv; rope mixes the two heads; scores = q'k'^T/32; softmax;
    out = (attn @ v) @ wout + bout

Sharding: core c owns (batch = c//2, seq-half = c%2) -> 1024 query rows.
Each core projects q/k/v for its own 1024 rows, ropes q/k, AllGathers
k'/v within the (2c, 2c+1) pair, runs attention for its rows against the
full 2048-seq k'/v, and applies the output projection for its rows.

Pipeline design (from perfetto traces of earlier revisions):
  * Weight streaming uses [128,16,512] slabs of wqkv (1KB DMA lines; the
    naive [128,16,128] stationary tiles produce 256B lines and halve
    effective DMA bandwidth).  All 12 slabs are emitted upfront on the
    sync queue in consumption order; a 5-deep ring paces them via WAR
    deps, so prefetch rides through the AllGather windows.
  * The pair AllGathers (k', v) are split in two 2MB chunks each,
    triggered at phase midpoints (k staging layout is permuted so the
    chunks are contiguous).
  * Attention k^T tiles live in the PERSIST pool: tiles allocated in the
    attn pool alias the projection pools' SBUF and inherit WAR deps
    until the end of q-proj, which would delay their DMA until
    attention start.  aoT/PT/wo are allocated first in the attn pool to
    absorb that aliased region (they are needed late).
  * Softmax denominators: DVE accumulates the 16 P^T chunks into one
    f32[128,512] partial, so only ONE ones-matmul per row-block runs on
    the PE (was 16).
  * x is loaded as four 1MB tiles so the first matmul chain starts
    after ~1MB (dep tracking is tile-granular).

On-device layouts (partition dim first):
    xT    [dim, rows]      - rhs/stationary for projections
    q'T   [2048, rows]     - head-dim on partitions (chunked [128,16,1024])
    k_in  [2048, rows]     - roped k^T, rows permuted: [h0 d0-511, h1 d0-511,
                             h0 d512-1023, h1 d512-1023] so each half is
                             contiguous for the chunked AllGather
    v     [rows, 2048]     - natural; gathered row-halves into v_g1/v_g2
    P^T   [seq_j, rows]    - exp(scores^T), bf16
    aoT   [2048, rows]     - unnormalized attn-out^T, normalized on write

Softmax skips max-subtraction: scores = q'.k'/32 ~ N(0,1), |scores| < ~8,
so exp is safe in f32 (verified against the reference distribution).
"""

import os
import sys

import numpy as np

if "/opt/trn_rl_repo" not in sys.path:
    sys.path.insert(0, "/opt/trn_rl_repo")

import ml_dtypes

# ---------------------------------------------------------------- constants
B, S, D = 4, 2048, 2048          # batch, seq, model dim
H, HD = 2, 1024                  # heads, head dim
R = 1024                         # query rows per core
N_CORES = 8
SCALE = 1.0 / 32.0               # HD ** -0.5

_NC_CACHE = {}
LAST_RESULT = {}

PAIRS = [[0, 1], [2, 3], [4, 5], [6, 7]]


def _build():
    import concourse.bass as bass
    import concourse.tile as tile
    from concourse import bacc, mybir

    F32 = mybir.dt.float32
    F16 = mybir.dt.float16
    BF = mybir.dt.bfloat16
    Exp = mybir.ActivationFunctionType.Exp

    NRB = R // 512                             # 512-row blocks for q / attention

    nc = bacc.Bacc("TRN2", target_bir_lowering=False, debug=False,
                   num_devices=N_CORES)

    xT = nc.dram_tensor("xT", [D, R], BF, kind="ExternalInput").ap()
    wqkv = nc.dram_tensor("wqkv", [D, 3 * D], BF, kind="ExternalInput").ap()
    wout = nc.dram_tensor("wout", [D, D], BF, kind="ExternalInput").ap()
    cost = nc.dram_tensor("cost", [512, R], F16, kind="ExternalInput").ap()
    sint = nc.dram_tensor("sint", [512, R], F16, kind="ExternalInput").ap()
    bias = nc.dram_tensor("bias", [1, D], F32, kind="ExternalInput").ap()
    out = nc.dram_tensor("out", [R, D], F32, kind="ExternalOutput").ap()

    wq_r = wqkv.rearrange("(c p) m -> p c m", p=128)    # [128, 16, 6144]
    x_r = xT.rearrange("(c p) r -> p c r", p=128)       # [128, 16, R]
    c_r = cost.rearrange("(c p) r -> p c r", p=128)     # [128, 4, R]
    s_r = sint.rearrange("(c p) r -> p c r", p=128)

    def bcast_ap(src_ap, nparts, width):
        return bass.AP(tensor=src_ap.tensor, offset=src_ap.offset,
                       ap=[[0, nparts], [1, width]])

    with tile.TileContext(nc) as tc:
        with (
            tc.tile_pool(name="persist", bufs=1) as persist,
            tc.tile_pool(name="psum", bufs=6, space="PSUM") as psp,
            tc.tile_pool(name="dram", bufs=1, space="DRAM") as dram,
        ):
            # ------------------------------------------- persistent buffers
            qT_sb = persist.tile([128, 16, R], BF, tag="qT")
            bias_sb = persist.tile([128, D], BF, tag="bias")
            ones_sb = persist.tile([128, 1], BF, tag="ones")
            # attention k^T halves (head dims 0-511 / 512-1023), reused for
            # both heads; persist-pool addresses never alias the projection
            # pools, so the head-0 loads run as soon as the gathers land.
            kTa = persist.tile([128, 4, S], BF, tag="kTa")
            kTb = persist.tile([128, 4, S], BF, tag="kTb")
            nc.vector.memset(ones_sb, 1.0)

            # DRAM scratch.  k_in rows permuted: row' = half*1024 + head*512
            # + (d % 512) for head-dim d, so halves are contiguous slabs.
            k_in = dram.tile([D, R], BF, tag="k_in")
            v_in = dram.tile([R, D], BF, tag="v_in")
            # gathered halves: [rank0 slab, rank1 slab]
            k_g1 = dram.tile([D, R], BF, tag="k_g1")   # d 0-511 both heads
            k_g2 = dram.tile([D, R], BF, tag="k_g2")   # d 512-1023 both heads
            v_g1 = dram.tile([S // 2, D], BF, tag="v_g1")  # local rows 0-511
            v_g2 = dram.tile([S // 2, D], BF, tag="v_g2")  # local rows 512-1023

            # =================================================== projections
            with (
                tc.tile_pool(name="proj", bufs=1) as proj,
                tc.tile_pool(name="projs", bufs=1) as projs,
            ):
                # x as four 1MB tiles on the scalar queue (starts earliest);
                # the K slabs head the sync queue so both stream in parallel
                x_parts = []
                for xp in range(4):
                    xt = proj.tile([128, 4, R], BF, tag="x", bufs=4)
                    nc.scalar.dma_start(out=xt, in_=x_r[:, xp * 4:(xp + 1) * 4, :])
                    x_parts.append(xt)

                def x_ap(kc, rs):
                    return x_parts[kc // 4][:, kc % 4, rs]

                # rope tables on gpsimd, chunk 0 first (needed by first rope)
                cs_sb = proj.tile([128, 4, R], F16, tag="cs")
                ss_sb = proj.tile([128, 4, R], F16, tag="ss")
                nc.gpsimd.dma_start(out=cs_sb[:, 0, :], in_=c_r[:, 0, :])
                nc.gpsimd.dma_start(out=ss_sb[:, 0, :], in_=s_r[:, 0, :])
                nc.gpsimd.dma_start(out=cs_sb[:, 1:, :], in_=c_r[:, 1:, :])
                nc.gpsimd.dma_start(out=ss_sb[:, 1:, :], in_=s_r[:, 1:, :])

                # ---- the 12 weight slabs, emitted upfront in consumption
                # order; the 5-deep ring paces the stream via WAR deps.
                # col bases: k head0 lo/hi = D+0 / D+512, k head1 = D+1024/+1536,
                # v = 2D + 512*vc, q analogous to k at base 0.
                slab_cols = [
                    D + 0, D + 1024,            # K0 (h0 lo), K2 (h1 lo)
                    D + 512, D + 1536,          # K1 (h0 hi), K3 (h1 hi)
                    2 * D + 0, 2 * D + 512, 2 * D + 1024, 2 * D + 1536,  # V0-3
                    0, 1024, 512, 1536,         # Q0, Q2, Q1, Q3
                ]
                slabs = {}
                for col0 in slab_cols:
                    sl = projs.tile([128, 16, 512], BF, tag="wsl", bufs=4)
                    for hh in (0, 8):
                        nc.sync.dma_start(
                            out=sl[:, hh:hh + 8, :],
                            in_=wq_r[:, hh:hh + 8, col0:col0 + 512])
                    slabs[col0] = sl

                def slab_w(base, c):
                    """[128,16,128] stationary slice for col chunk c of the
                    2048-wide projection starting at wqkv col `base`."""
                    col = base + c * 128
                    sl = slabs[base + (col - base) // 512 * 512]
                    return sl[:, :, (c % 4) * 128:(c % 4 + 1) * 128]

                def qk_proj(col0, emit, on_c_done=None):
                    """Project+rope cols [col0, col0+2048) of wqkv.

                    emit(c, rb, apA, apB): receive bf16 [128,512] rope outputs
                    for col-chunk c (head0) and c+8 (head1), row block rb."""
                    for c in range(8):
                        wa = slab_w(col0, c)
                        wb = slab_w(col0 + 1024, c)
                        w1s = lambda kc, wa=wa: wa[:, kc, :]
                        w2s = lambda kc, wb=wb: wb[:, kc, :]
                        for rb in range(NRB):
                            rs = slice(rb * 512, (rb + 1) * 512)
                            ps1 = psp.tile([128, 512], F32, tag="mm")
                            ps2 = psp.tile([128, 512], F32, tag="mm")
                            for kc in range(16):
                                nc.tensor.matmul(ps1, w1s(kc), x_ap(kc, rs),
                                                 start=kc == 0, stop=kc == 15)
                            for kc in range(16):
                                nc.tensor.matmul(ps2, w2s(kc), x_ap(kc, rs),
                                                 start=kc == 0, stop=kc == 15)
                            cosv = cs_sb[:, c % 4, rs]
                            sinv = ss_sb[:, c % 4, rs]
                            t1 = projs.tile([128, 512], F32, tag="rt", bufs=4)
                            t2 = projs.tile([128, 512], F32, tag="rt", bufs=4)
                            outA = projs.tile([128, 512], BF, tag="ro", bufs=4)
                            outB = projs.tile([128, 512], BF, tag="ro", bufs=4)
                            nc.vector.tensor_mul(t1, ps1, cosv)
                            nc.vector.tensor_mul(t2, ps2, sinv)
                            nc.vector.tensor_sub(outA, t1, t2)
                            nc.vector.tensor_mul(t1, ps2, cosv)
                            nc.vector.tensor_mul(t2, ps1, sinv)
                            nc.vector.tensor_add(outB, t1, t2)
                            emit(c, rb, outA, outB)
                        if on_c_done is not None:
                            on_c_done(c)

                # ---- k projection + rope -> permuted k_in, chunked AllGather
                def emit_k(c, rb, apA, apB):
                    rs = slice(rb * 512, (rb + 1) * 512)
                    half, cc = c // 4, c % 4
                    base = half * 1024
                    nc.gpsimd.dma_start(
                        out=k_in[base + cc * 128:base + (cc + 1) * 128, rs],
                        in_=apA)
                    nc.gpsimd.dma_start(
                        out=k_in[base + 512 + cc * 128:base + 512 + (cc + 1) * 128, rs],
                        in_=apB)

                def k_ag(c):
                    if c == 3:
                        nc.gpsimd.collective_compute(
                            "AllGather", bass.mybir.AluOpType.bypass,
                            replica_groups=PAIRS,
                            ins=[k_in[0:1024, :].opt()], outs=[k_g1.opt()])
                    elif c == 7:
                        nc.gpsimd.collective_compute(
                            "AllGather", bass.mybir.AluOpType.bypass,
                            replica_groups=PAIRS,
                            ins=[k_in[1024:2048, :].opt()], outs=[k_g2.opt()])

                qk_proj(D, emit_k, on_c_done=k_ag)

                # ---- v projection (natural layout), row-half outer so each
                # half AllGathers while the other half computes
                for half in range(2):
                    for vc in range(4):
                        wv = slabs[2 * D + vc * 512]
                        for rr in range(half * 4, half * 4 + 4):
                            ps = psp.tile([128, 512], F32, tag="mm")
                            for kc in range(16):
                                nc.tensor.matmul(
                                    ps, x_ap(kc, slice(rr * 128, (rr + 1) * 128)),
                                    wv[:, kc, :], start=kc == 0, stop=kc == 15)
                            vt = projs.tile([128, 512], BF, tag="vo", bufs=10)
                            nc.scalar.copy(vt, ps)
                            nc.scalar.dma_start(
                                out=v_in[rr * 128:(rr + 1) * 128,
                                         vc * 512:(vc + 1) * 512],
                                in_=vt)
                    vg = v_g1 if half == 0 else v_g2
                    nc.gpsimd.collective_compute(
                        "AllGather", bass.mybir.AluOpType.bypass,
                        replica_groups=PAIRS,
                        ins=[v_in[half * 512:(half + 1) * 512, :].opt()],
                        outs=[vg.opt()])

                # ---- q projection + rope -> qT_sb (resident)
                def emit_q(c, rb, apA, apB):
                    rs = slice(rb * 512, (rb + 1) * 512)
                    nc.vector.tensor_copy(qT_sb[:, c, rs], apA)
                    nc.vector.tensor_copy(qT_sb[:, c + 8, rs], apB)

                qk_proj(0, emit_q)

            # ====================================== attention + output proj
            # Allocation order is load-bearing: the attn pool reuses the
            # closed proj pools' SBUF region, which carries WAR deps until
            # the end of q-proj.  aoT/PT/wo (all needed late) absorb it.
            with tc.tile_pool(name="attn", bufs=1) as attn:
                aoT_sb = attn.tile([128, 16, R], BF, tag="aoT")
                PT = attn.tile([128, 16, 512], BF, tag="PT")
                wout_r = wout.rearrange("(c p) m -> p c m", p=128)
                wo_ring = []
                for wi in range(2):
                    wo_t = attn.tile([128, 16, 256], BF, tag="wo", bufs=2)
                    for dc in range(0, 16, 4):
                        nc.scalar.dma_start(
                            out=wo_t[:, dc:dc + 4, :],
                            in_=wout_r[:, dc:dc + 4, wi * 256:(wi + 1) * 256])
                    wo_ring.append(wo_t)
                nc.gpsimd.dma_start(out=bias_sb, in_=bcast_ap(bias, 128, D))
                for hi in range(H):
                    for sh in range(2):
                        nc.scalar.dma_start(
                            out=kTa[:, :, sh * R:(sh + 1) * R],
                            in_=k_g1[sh * 1024 + hi * 512:sh * 1024 + (hi + 1) * 512,
                                     :].rearrange("(c p) r -> p c r", p=128))
                        nc.scalar.dma_start(
                            out=kTb[:, :, sh * R:(sh + 1) * R],
                            in_=k_g2[sh * 1024 + hi * 512:sh * 1024 + (hi + 1) * 512,
                                     :].rearrange("(c p) r -> p c r", p=128))
                    # v rows for this head: global key chunks jc 0..15 map to
                    # [v_g1 sh0, v_g2 sh0, v_g1 sh1, v_g2 sh1] quarters.
                    v_sb = attn.tile([128, 16, HD], BF, tag="vh")
                    for quarter in range(4):
                        vg = v_g1 if quarter % 2 == 0 else v_g2
                        sh = quarter // 2
                        nc.scalar.dma_start(
                            out=v_sb[:, quarter * 4:(quarter + 1) * 4, :],
                            in_=vg[sh * 512:(sh + 1) * 512,
                                   hi * HD:(hi + 1) * HD].rearrange(
                                "(c p) m -> p c m", p=128))
                    for rb in range(NRB):
                        rs = slice(rb * 512, (rb + 1) * 512)
                        acc = None
                        for jc in range(16):
                            ps = psp.tile([128, 512], F32, tag="mm")
                            for dc in range(8):
                                kt = kTa if dc < 4 else kTb
                                nc.tensor.matmul(
                                    ps, kt[:, dc % 4, jc * 128:(jc + 1) * 128],
                                    qT_sb[:, hi * 8 + dc, rs],
                                    start=dc == 0, stop=dc == 7)
                            nc.scalar.activation(PT[:, jc, :], ps, Exp, scale=SCALE)
                            # DVE accumulates the denominator partials so the
                            # PE only runs ONE ones-matmul per row block
                            if jc == 1:
                                a0 = attn.tile([128, 512], F32, tag="sacc", bufs=2)
                                nc.vector.tensor_add(a0, PT[:, 0, :], PT[:, 1, :])
                                acc = a0
                            elif jc > 1:
                                if jc < 15:
                                    a1 = attn.tile([128, 512], F32, tag="sacc", bufs=2)
                                else:
                                    a1 = attn.tile([128, 512], BF, tag="sbf", bufs=2)
                                nc.vector.tensor_add(a1, acc, PT[:, jc, :])
                                acc = a1
                        sps = psp.tile([1, 512], F32, tag="sum", bufs=1)
                        nc.tensor.matmul(sps, ones_sb, acc, start=True, stop=True)
                        rec = attn.tile([1, 512], F32, tag="rec", bufs=2)
                        nc.vector.reciprocal(rec, sps)
                        rec_d = dram.tile([1, 512], F32, tag="rec_d", bufs=2)
                        nc.sync.dma_start(out=rec_d, in_=rec)
                        rec_b = attn.tile([128, 512], F32, tag="rec_b", bufs=2)
                        nc.sync.dma_start(out=rec_b, in_=bcast_ap(rec_d, 128, 512))
                        for m in range(8):
                            pa = psp.tile([128, 512], F32, tag="mm")
                            for jc in range(16):
                                nc.tensor.matmul(
                                    pa, v_sb[:, jc, m * 128:(m + 1) * 128],
                                    PT[:, jc, :], start=jc == 0, stop=jc == 15)
                            nc.vector.tensor_mul(aoT_sb[:, hi * 8 + m, rs], pa, rec_b)

                # ---------------------------------------- output projection
                for cc in range(8):
                    wo = wo_ring[cc % 2]
                    if cc >= 2:
                        for dc in range(0, 16, 4):
                            nc.scalar.dma_start(
                                out=wo[:, dc:dc + 4, :],
                                in_=wout_r[:, dc:dc + 4, cc * 256:(cc + 1) * 256])
                    for rr in range(R // 128):
                        r0 = rr * 128
                        ps = psp.tile([128, 256], F32, tag="mm")
                        for dc in range(16):
                            nc.tensor.matmul(ps, aoT_sb[:, dc, r0:r0 + 128],
                                             wo[:, dc, :],
                                             start=dc == 0, stop=dc == 15)
                        ot = attn.tile([128, 256], F32, tag="ot", bufs=2)
                        nc.vector.tensor_add(ot, ps, bias_sb[:, cc * 256:(cc + 1) * 256])
                        nc.gpsimd.dma_start(
                            out=out[r0:r0 + 128, cc * 256:(cc + 1) * 256], in_=ot)

    nc.compile()
    return nc


def _get_nc():
    if "v5" not in _NC_CACHE:
        _NC_CACHE["v5"] = _build()
    return _NC_CACHE["v5"]


def _rope_tables():
    inv_freq = 1.0 / (10000.0 ** (np.arange(0, HD, 2, dtype=np.float32) / HD))
    t = np.arange(S, dtype=np.float32)
    freqs = t[:, None] * inv_freq[None, :]          # (S, 512)
    return np.cos(freqs).astype(np.float32), np.sin(freqs).astype(np.float32)


def kernel(x, wqkv, wout, bout):
    from concourse.bass_utils import run_bass_kernel_spmd

    bf16 = ml_dtypes.bfloat16
    x = np.asarray(x, dtype=np.float32)
    wqkv_b = np.ascontiguousarray(np.asarray(wqkv, dtype=np.float32)).astype(bf16)
    wout_b = np.ascontiguousarray(np.asarray(wout, dtype=np.float32)).astype(bf16)
    bout_f = np.asarray(bout, dtype=np.float32).reshape(1, D)
    cos_h, sin_h = _rope_tables()                   # (S, 512) f32
    cosT = np.ascontiguousarray(cos_h.T)            # (512, S)
    sinT = np.ascontiguousarray(sin_h.T)

    nc = _get_nc()

    in_maps = []
    for c in range(N_CORES):
        bi, half = c // 2, c % 2
        rows = slice(half * R, (half + 1) * R)
        xT_own = np.ascontiguousarray(x[bi, rows, :].T).astype(bf16)
        in_maps.append({
            "wqkv": wqkv_b,
            "wout": wout_b,
            "bias": bout_f,
            "xT": xT_own,
            "cost": np.ascontiguousarray(cosT[:, rows]).astype(np.float16),
            "sint": np.ascontiguousarray(sinT[:, rows]).astype(np.float16),
        })

    trace = os.environ.get("KERNEL_TRACE", "0") == "1"
    res = run_bass_kernel_spmd(nc, in_maps, list(range(N_CORES)), trace=trace)
    if trace:
        LAST_RESULT["exec_time_ns"] = res.exec_time_ns
        LAST_RESULT["mean_exec_time_ns"] = res.mean_exec_time_ns

    out_full = np.empty((B, S, D), np.float32)
    for c in range(N_CORES):
        bi, half = c // 2, c % 2
        out_full[bi, half * R:(half + 1) * R, :] = res.results[c]["out"]
    return out_full


# revision 20
# speedup vs baseline: 1.0092x; 1.0092x over previous
"""Fused multi-head attention (2 heads, RoPE-across-heads) on 8 Trainium2 NeuronCores.

Reference computation (per batch b of 4, seq 2048, dim 2048):
    qkv = x @ wqkv; rope mixes the two heads; scores = q'k'^T/32; softmax;
    out = (attn @ v) @ wout + bout

Sharding: core c owns (batch = c//2, seq-half = c%2) -> 1024 query rows.
Each core projects q/k/v for its own 1024 rows, ropes q/k, AllGathers
k'/v within the (2c, 2c+1) pair, runs attention for its rows against the
full 2048-seq k'/v, and applies the output projection for its rows.

Pipeline design (from perfetto traces of earlier revisions):
  * Weight streaming uses [128,16,512] slabs of wqkv (1KB DMA lines; the
    naive [128,16,128] stationary tiles produce 256B lines and halve
    effective DMA bandwidth).  All 12 slabs are emitted upfront on the
    sync queue in consumption order; a 5-deep ring paces them via WAR
    deps, so prefetch rides through the AllGather windows.
  * The pair AllGathers (k', v) are split in two 2MB chunks each,
    triggered at phase midpoints (k staging layout is permuted so the
    chunks are contiguous).
  * Attention k^T tiles live in the PERSIST pool: tiles allocated in the
    attn pool alias the projection pools' SBUF and inherit WAR deps
    until the end of q-proj, which would delay their DMA until
    attention start.  aoT/PT/wo are allocated first in the attn pool to
    absorb that aliased region (they are needed late).
  * Softmax denominators: DVE accumulates the 16 P^T chunks into one
    f32[128,512] partial, so only ONE ones-matmul per row-block runs on
    the PE (was 16).
  * x is loaded as four 1MB tiles so the first matmul chain starts
    after ~1MB (dep tracking is tile-granular).

On-device layouts (partition dim first):
    xT    [dim, rows]      - rhs/stationary for projections
    q'T   [2048, rows]     - head-dim on partitions (chunked [128,16,1024])
    k_in  [2048, rows]     - roped k^T, rows permuted: [h0 d0-511, h1 d0-511,
                             h0 d512-1023, h1 d512-1023] so each half is
                             contiguous for the chunked AllGather
    v     [rows, 2048]     - natural; gathered row-halves into v_g1/v_g2
    P^T   [seq_j, rows]    - exp(scores^T), bf16
    aoT   [2048, rows]     - unnormalized attn-out^T, normalized on write

Softmax skips max-subtraction: scores = q'.k'/32 ~ N(0,1), |scores| < ~8,
so exp is safe in f32 (verified against the reference distribution).
"""

import os
import sys

import numpy as np

if "/opt/trn_rl_repo" not in sys.path:
    sys.path.insert(0, "/opt/trn_rl_repo")

import ml_dtypes

# ---------------------------------------------------------------- constants
B, S, D = 4, 2048, 2048          # batch, seq, model dim
H, HD = 2, 1024                  # heads, head dim
R = 1024                         # query rows per core
N_CORES = 8
SCALE = 1.0 / 32.0               # HD ** -0.5

_NC_CACHE = {}
LAST_RESULT = {}

PAIRS = [[0, 1], [2, 3], [4, 5], [6, 7]]


def _build():
    import concourse.bass as bass
    import concourse.tile as tile
    from concourse import bacc, mybir

    F32 = mybir.dt.float32
    F16 = mybir.dt.float16
    BF = mybir.dt.bfloat16
    Exp = mybir.ActivationFunctionType.Exp

    NRB = R // 512                             # 512-row blocks for q / attention

    nc = bacc.Bacc("TRN2", target_bir_lowering=False, debug=False,
                   num_devices=N_CORES)

    xT = nc.dram_tensor("xT", [D, R], BF, kind="ExternalInput").ap()
    wqkv = nc.dram_tensor("wqkv", [D, 3 * D], BF, kind="ExternalInput").ap()
    wout = nc.dram_tensor("wout", [D, D], BF, kind="ExternalInput").ap()
    cost = nc.dram_tensor("cost", [512, R], F16, kind="ExternalInput").ap()
    sint = nc.dram_tensor("sint", [512, R], F16, kind="ExternalInput").ap()
    bias = nc.dram_tensor("bias", [1, D], F32, kind="ExternalInput").ap()
    out = nc.dram_tensor("out", [R, D], F32, kind="ExternalOutput").ap()

    wq_r = wqkv.rearrange("(c p) m -> p c m", p=128)    # [128, 16, 6144]
    x_r = xT.rearrange("(c p) r -> p c r", p=128)       # [128, 16, R]
    c_r = cost.rearrange("(c p) r -> p c r", p=128)     # [128, 4, R]
    s_r = sint.rearrange("(c p) r -> p c r", p=128)

    def bcast_ap(src_ap, nparts, width):
        return bass.AP(tensor=src_ap.tensor, offset=src_ap.offset,
                       ap=[[0, nparts], [1, width]])

    with tile.TileContext(nc) as tc:
        with (
            tc.tile_pool(name="persist", bufs=1) as persist,
            tc.tile_pool(name="psum", bufs=6, space="PSUM") as psp,
            tc.tile_pool(name="dram", bufs=1, space="DRAM") as dram,
        ):
            # ------------------------------------------- persistent buffers
            qT_sb = persist.tile([128, 16, R], BF, tag="qT")
            bias_sb = persist.tile([128, D], BF, tag="bias")
            ones_sb = persist.tile([128, 1], BF, tag="ones")
            # attention k^T halves (head dims 0-511 / 512-1023), reused for
            # both heads; persist-pool addresses never alias the projection
            # pools, so the head-0 loads run as soon as the gathers land.
            kTa = persist.tile([128, 4, S], BF, tag="kTa")
            kTb = persist.tile([128, 4, S], BF, tag="kTb")
            nc.vector.memset(ones_sb, 1.0)

            # DRAM scratch.  k_in rows permuted: row' = half*1024 + head*512
            # + (d % 512) for head-dim d, so halves are contiguous slabs.
            k_in = dram.tile([D, R], BF, tag="k_in")
            v_in = dram.tile([R, D], BF, tag="v_in")
            # gathered halves: [rank0 slab, rank1 slab]
            k_g1 = dram.tile([D, R], BF, tag="k_g1")   # d 0-511 both heads
            k_g2 = dram.tile([D, R], BF, tag="k_g2")   # d 512-1023 both heads
            v_g1 = dram.tile([S // 2, D], BF, tag="v_g1")  # local rows 0-511
            v_g2 = dram.tile([S // 2, D], BF, tag="v_g2")  # local rows 512-1023

            # =================================================== projections
            with (
                tc.tile_pool(name="proj", bufs=1) as proj,
                tc.tile_pool(name="projs", bufs=1) as projs,
            ):
                # x as four 1MB tiles heading the sync queue, ahead of the
                # weight slabs
                x_parts = []
                for xp in range(4):
                    xt = proj.tile([128, 4, R], BF, tag="x", bufs=4)
                    nc.sync.dma_start(out=xt, in_=x_r[:, xp * 4:(xp + 1) * 4, :])
                    x_parts.append(xt)

                def x_ap(kc, rs):
                    return x_parts[kc // 4][:, kc % 4, rs]

                # rope tables: chunk 0 first on scalar (needed by the first
                # rope), the rest on gpsimd off the weight stream's path
                cs_sb = proj.tile([128, 4, R], F16, tag="cs")
                ss_sb = proj.tile([128, 4, R], F16, tag="ss")
                nc.scalar.dma_start(out=cs_sb[:, 0, :], in_=c_r[:, 0, :])
                nc.scalar.dma_start(out=ss_sb[:, 0, :], in_=s_r[:, 0, :])
                nc.gpsimd.dma_start(out=cs_sb[:, 1:, :], in_=c_r[:, 1:, :])
                nc.gpsimd.dma_start(out=ss_sb[:, 1:, :], in_=s_r[:, 1:, :])

                # ---- the 12 weight slabs, emitted upfront in consumption
                # order; the 5-deep ring paces the stream via WAR deps.
                # col bases: k head0 lo/hi = D+0 / D+512, k head1 = D+1024/+1536,
                # v = 2D + 512*vc, q analogous to k at base 0.
                slab_cols = [
                    D + 0, D + 1024,            # K0 (h0 lo), K2 (h1 lo)
                    D + 512, D + 1536,          # K1 (h0 hi), K3 (h1 hi)
                    2 * D + 0, 2 * D + 512, 2 * D + 1024, 2 * D + 1536,  # V0-3
                    0, 1024, 512, 1536,         # Q0, Q2, Q1, Q3
                ]
                slabs = {}
                for col0 in slab_cols:
                    sl = projs.tile([128, 16, 512], BF, tag="wsl", bufs=4)
                    for hh in (0, 8):
                        nc.sync.dma_start(
                            out=sl[:, hh:hh + 8, :],
                            in_=wq_r[:, hh:hh + 8, col0:col0 + 512])
                    slabs[col0] = sl

                def slab_w(base, c):
                    """[128,16,128] stationary slice for col chunk c of the
                    2048-wide projection starting at wqkv col `base`."""
                    col = base + c * 128
                    sl = slabs[base + (col - base) // 512 * 512]
                    return sl[:, :, (c % 4) * 128:(c % 4 + 1) * 128]

                def qk_proj(col0, emit, on_c_done=None):
                    """Project+rope cols [col0, col0+2048) of wqkv.

                    emit(c, rb, apA, apB): receive bf16 [128,512] rope outputs
                    for col-chunk c (head0) and c+8 (head1), row block rb."""
                    for c in range(8):
                        wa = slab_w(col0, c)
                        wb = slab_w(col0 + 1024, c)
                        w1s = lambda kc, wa=wa: wa[:, kc, :]
                        w2s = lambda kc, wb=wb: wb[:, kc, :]
                        for rb in range(NRB):
                            rs = slice(rb * 512, (rb + 1) * 512)
                            ps1 = psp.tile([128, 512], F32, tag="mm")
                            ps2 = psp.tile([128, 512], F32, tag="mm")
                            for kc in range(16):
                                nc.tensor.matmul(ps1, w1s(kc), x_ap(kc, rs),
                                                 start=kc == 0, stop=kc == 15)
                            for kc in range(16):
                                nc.tensor.matmul(ps2, w2s(kc), x_ap(kc, rs),
                                                 start=kc == 0, stop=kc == 15)
                            cosv = cs_sb[:, c % 4, rs]
                            sinv = ss_sb[:, c % 4, rs]
                            t1 = projs.tile([128, 512], F32, tag="rt", bufs=4)
                            t2 = projs.tile([128, 512], F32, tag="rt", bufs=4)
                            outA = projs.tile([128, 512], BF, tag="ro", bufs=4)
                            outB = projs.tile([128, 512], BF, tag="ro", bufs=4)
                            nc.vector.tensor_mul(t1, ps1, cosv)
                            nc.vector.tensor_mul(t2, ps2, sinv)
                            nc.vector.tensor_sub(outA, t1, t2)
                            nc.vector.tensor_mul(t1, ps2, cosv)
                            nc.vector.tensor_mul(t2, ps1, sinv)
                            nc.vector.tensor_add(outB, t1, t2)
                            emit(c, rb, outA, outB)
                        if on_c_done is not None:
                            on_c_done(c)

                # ---- k projection + rope -> permuted k_in, chunked AllGather
                def emit_k(c, rb, apA, apB):
                    rs = slice(rb * 512, (rb + 1) * 512)
                    half, cc = c // 4, c % 4
                    base = half * 1024
                    nc.gpsimd.dma_start(
                        out=k_in[base + cc * 128:base + (cc + 1) * 128, rs],
                        in_=apA)
                    nc.gpsimd.dma_start(
                        out=k_in[base + 512 + cc * 128:base + 512 + (cc + 1) * 128, rs],
                        in_=apB)

                def k_ag(c):
                    if c == 3:
                        nc.gpsimd.collective_compute(
                            "AllGather", bass.mybir.AluOpType.bypass,
                            replica_groups=PAIRS,
                            ins=[k_in[0:1024, :].opt()], outs=[k_g1.opt()])
                    elif c == 7:
                        nc.gpsimd.collective_compute(
                            "AllGather", bass.mybir.AluOpType.bypass,
                            replica_groups=PAIRS,
                            ins=[k_in[1024:2048, :].opt()], outs=[k_g2.opt()])

                qk_proj(D, emit_k, on_c_done=k_ag)

                # ---- v projection (natural layout), row-half outer so each
                # half AllGathers while the other half computes
                for half in range(2):
                    for vc in range(4):
                        wv = slabs[2 * D + vc * 512]
                        for rr in range(half * 4, half * 4 + 4):
                            ps = psp.tile([128, 512], F32, tag="mm")
                            for kc in range(16):
                                nc.tensor.matmul(
                                    ps, x_ap(kc, slice(rr * 128, (rr + 1) * 128)),
                                    wv[:, kc, :], start=kc == 0, stop=kc == 15)
                            vt = projs.tile([128, 512], BF, tag="vo", bufs=10)
                            nc.scalar.copy(vt, ps)
                            nc.scalar.dma_start(
                                out=v_in[rr * 128:(rr + 1) * 128,
                                         vc * 512:(vc + 1) * 512],
                                in_=vt)
                    vg = v_g1 if half == 0 else v_g2
                    nc.gpsimd.collective_compute(
                        "AllGather", bass.mybir.AluOpType.bypass,
                        replica_groups=PAIRS,
                        ins=[v_in[half * 512:(half + 1) * 512, :].opt()],
                        outs=[vg.opt()])

                # ---- q projection + rope -> qT_sb (resident)
                def emit_q(c, rb, apA, apB):
                    rs = slice(rb * 512, (rb + 1) * 512)
                    nc.vector.tensor_copy(qT_sb[:, c, rs], apA)
                    nc.vector.tensor_copy(qT_sb[:, c + 8, rs], apB)

                qk_proj(0, emit_q)

            # ====================================== attention + output proj
            # Allocation order is load-bearing: the attn pool reuses the
            # closed proj pools' SBUF region, which carries WAR deps until
            # the end of q-proj.  aoT/PT/wo (all needed late) absorb it.
            with tc.tile_pool(name="attn", bufs=1) as attn:
                aoT_sb = attn.tile([128, 16, R], BF, tag="aoT")
                PT = attn.tile([128, 16, 512], BF, tag="PT")
                wout_r = wout.rearrange("(c p) m -> p c m", p=128)
                wo_ring = []
                for wi in range(2):
                    wo_t = attn.tile([128, 16, 256], BF, tag="wo", bufs=2)
                    for dc in range(0, 16, 4):
                        nc.scalar.dma_start(
                            out=wo_t[:, dc:dc + 4, :],
                            in_=wout_r[:, dc:dc + 4, wi * 256:(wi + 1) * 256])
                    wo_ring.append(wo_t)
                nc.gpsimd.dma_start(out=bias_sb, in_=bcast_ap(bias, 128, D))
                for hi in range(H):
                    for sh in range(2):
                        nc.scalar.dma_start(
                            out=kTa[:, :, sh * R:(sh + 1) * R],
                            in_=k_g1[sh * 1024 + hi * 512:sh * 1024 + (hi + 1) * 512,
                                     :].rearrange("(c p) r -> p c r", p=128))
                        nc.scalar.dma_start(
                            out=kTb[:, :, sh * R:(sh + 1) * R],
                            in_=k_g2[sh * 1024 + hi * 512:sh * 1024 + (hi + 1) * 512,
                                     :].rearrange("(c p) r -> p c r", p=128))
                    # v rows for this head: global key chunks jc 0..15 map to
                    # [v_g1 sh0, v_g2 sh0, v_g1 sh1, v_g2 sh1] quarters.
                    v_sb = attn.tile([128, 16, HD], BF, tag="vh")
                    for quarter in range(4):
                        vg = v_g1 if quarter % 2 == 0 else v_g2
                        sh = quarter // 2
                        nc.scalar.dma_start(
                            out=v_sb[:, quarter * 4:(quarter + 1) * 4, :],
                            in_=vg[sh * 512:(sh + 1) * 512,
                                   hi * HD:(hi + 1) * HD].rearrange(
                                "(c p) m -> p c m", p=128))
                    for rb in range(NRB):
                        rs = slice(rb * 512, (rb + 1) * 512)
                        acc = None
                        for jc in range(16):
                            ps = psp.tile([128, 512], F32, tag="mm")
                            for dc in range(8):
                                kt = kTa if dc < 4 else kTb
                                nc.tensor.matmul(
                                    ps, kt[:, dc % 4, jc * 128:(jc + 1) * 128],
                                    qT_sb[:, hi * 8 + dc, rs],
                                    start=dc == 0, stop=dc == 7)
                            nc.scalar.activation(PT[:, jc, :], ps, Exp, scale=SCALE)
                            # DVE accumulates the denominator partials so the
                            # PE only runs ONE ones-matmul per row block
                            if jc == 1:
                                a0 = attn.tile([128, 512], F32, tag="sacc", bufs=2)
                                nc.vector.tensor_add(a0, PT[:, 0, :], PT[:, 1, :])
                                acc = a0
                            elif jc > 1:
                                if jc < 15:
                                    a1 = attn.tile([128, 512], F32, tag="sacc", bufs=2)
                                else:
                                    a1 = attn.tile([128, 512], BF, tag="sbf", bufs=2)
                                nc.vector.tensor_add(a1, acc, PT[:, jc, :])
                                acc = a1
                        sps = psp.tile([1, 512], F32, tag="sum", bufs=1)
                        nc.tensor.matmul(sps, ones_sb, acc, start=True, stop=True)
                        rec = attn.tile([1, 512], F32, tag="rec", bufs=2)
                        nc.vector.reciprocal(rec, sps)
                        rec_d = dram.tile([1, 512], F32, tag="rec_d", bufs=2)
                        nc.sync.dma_start(out=rec_d, in_=rec)
                        rec_b = attn.tile([128, 512], F32, tag="rec_b", bufs=2)
                        nc.sync.dma_start(out=rec_b, in_=bcast_ap(rec_d, 128, 512))
                        for m in range(8):
                            pa = psp.tile([128, 512], F32, tag="mm")
                            for jc in range(16):
                                nc.tensor.matmul(
                                    pa, v_sb[:, jc, m * 128:(m + 1) * 128],
                                    PT[:, jc, :], start=jc == 0, stop=jc == 15)
                            nc.vector.tensor_mul(aoT_sb[:, hi * 8 + m, rs], pa, rec_b)

                # ---------------------------------------- output projection
                for cc in range(8):
                    wo = wo_ring[cc % 2]
                    if cc >= 2:
                        for dc in range(0, 16, 4):
                            nc.scalar.dma_start(
                                out=wo[:, dc:dc + 4, :],
                                in_=wout_r[:, dc:dc + 4, cc * 256:(cc + 1) * 256])
                    for rr in range(R // 128):
                        r0 = rr * 128
                        ps = psp.tile([128, 256], F32, tag="mm")
                        for dc in range(16):
                            nc.tensor.matmul(ps, aoT_sb[:, dc, r0:r0 + 128],
                                             wo[:, dc, :],
                                             start=dc == 0, stop=dc == 15)
                        ot = attn.tile([128, 256], F32, tag="ot", bufs=2)
                        nc.vector.tensor_add(ot, ps, bias_sb[:, cc * 256:(cc + 1) * 256])
                        nc.gpsimd.dma_start(
                            out=out[r0:r0 + 128, cc * 256:(cc + 1) * 256], in_=ot)

    nc.compile()
    return nc


def _get_nc():
    if "v5" not in _NC_CACHE:
        _NC_CACHE["v5"] = _build()
    return _NC_CACHE["v5"]


def _rope_tables():
    inv_freq = 1.0 / (10000.0 ** (np.arange(0, HD, 2, dtype=np.float32) / HD))
    t = np.arange(S, dtype=np.float32)
    freqs = t[:, None] * inv_freq[None, :]          # (S, 512)
    return np.cos(freqs).astype(np.float32), np.sin(freqs).astype(np.float32)


def kernel(x, wqkv, wout, bout):
    from concourse.bass_utils import run_bass_kernel_spmd

    bf16 = ml_dtypes.bfloat16
    x = np.asarray(x, dtype=np.float32)
    wqkv_b = np.ascontiguousarray(np.asarray(wqkv, dtype=np.float32)).astype(bf16)
    wout_b = np.ascontiguousarray(np.asarray(wout, dtype=np.float32)).astype(bf16)
    bout_f = np.asarray(bout, dtype=np.float32).reshape(1, D)
    cos_h, sin_h = _rope_tables()                   # (S, 512) f32
    cosT = np.ascontiguousarray(cos_h.T)            # (512, S)
    sinT = np.ascontiguousarray(sin_h.T)

    nc = _get_nc()

    in_maps = []
    for c in range(N_CORES):
        bi, half = c // 2, c % 2
        rows = slice(half * R, (half + 1) * R)
        xT_own = np.ascontiguousarray(x[bi, rows, :].T).astype(bf16)
        in_maps.append({
            "wqkv": wqkv_b,
            "wout": wout_b,
            "bias": bout_f,
            "xT": xT_own,
            "cost": np.ascontiguousarray(cosT[:, rows]).astype(np.float16),
            "sint": np.ascontiguousarray(sinT[:, rows]).astype(np.float16),
        })

    trace = os.environ.get("KERNEL_TRACE", "0") == "1"
    res = run_bass_kernel_spmd(nc, in_maps, list(range(N_CORES)), trace=trace)
    if trace:
        LAST_RESULT["exec_time_ns"] = res.exec_time_ns
        LAST_RESULT["mean_exec_time_ns"] = res.mean_exec_time_ns

    out_full = np.empty((B, S, D), np.float32)
    for c in range(N_CORES):
        bi, half = c // 2, c % 2
        out_full[bi, half * R:(half + 1) * R, :] = res.results[c]["out"]
    return out_full
